# revision 26
# baseline (speedup 1.0000x reference)
import sys
for _p in ("/opt/trn_rl_repo",):
    if _p not in sys.path:
        sys.path.insert(0, _p)
"""Causal self-attention TRN2 kernel, 8-core SPMD — fused pipeline v2.

Sharding: core c handles batch b = c//2 and head-half h = c%2 (8 of 16 heads,
i.e. 512 of 1024 q/k/v channels). Host sums the two output-projection partials
per batch and concatenates batches.

v2 layout (vs v1 phase-separated):
  - x resident in SBUF whole kernel; V computed first (dense PE warm-up for
    the HAM clock gate), then per channel-group g (= head pair): q/k projected
    straight into SBUF (no DRAM bounce), attention for g immediately after.
  - Projection matmuls for group g+1 are interleaved into group g's attention
    j-loop so the PE has dense work while ACT (exp) paces the softmax.
  - hh processed serially (not tile-position-packed) to fit PSUM in 8 banks:
    S double-buffer 2x[128,1024] (4) + y accum 2x[65,512] (2) + shared
    accumulation ring 2x[128,512] (2).
  - Softmax denominators via ones-column in V (y row 64); per-g batched
    reciprocal bounced through DRAM (cross-partition gather), normalize
    overlapped under the next group's attention.
"""

import numpy as np
import concourse.bass as bass
import concourse.mybir as mybir
import concourse.tile as tile
from concourse import bacc

P = 128
T = 2048          # tokens per batch
F = 1024          # model dim (projection contraction)
CH = 512          # per-core q/k/v channels (8 heads x 64)
NH = 8            # heads per core
D = 64
NKO = F // P      # 8
NM = CH // P      # 4 channel tiles (= head pairs)
GSZ = 512         # tq group size
NG = T // GSZ     # 4
NTT = T // P      # 16
OUTC = 1024
NCHAIN = NM * NG * 2   # 32 softmax-denominator chains

f32 = mybir.dt.float32
f32r = mybir.dt.float32r
bf16 = mybir.dt.bfloat16


def build_nc(dt_mode: str = "bf16"):
    """Build the per-core Bass program. dt_mode in {"f32", "f32r", "bf16"}."""
    din = {"f32": f32, "f32r": f32r, "bf16": bf16}[dt_mode]
    ddram = din

    nc = bacc.Bacc("TRN2", target_bir_lowering=False, debug=False, num_devices=8)

    xT_d = nc.declare_dram_parameter("xT", [F, T], ddram, isOutput=False)
    wqT_d = nc.declare_dram_parameter("wqT", [F, CH], ddram, isOutput=False)
    wkT_d = nc.declare_dram_parameter("wkT", [F, CH], ddram, isOutput=False)
    wvT_d = nc.declare_dram_parameter("wvT", [F, CH], ddram, isOutput=False)
    wpT_d = nc.declare_dram_parameter("wpT", [CH, OUTC], ddram, isOutput=False)
    bq_d = nc.declare_dram_parameter("bq", [CH], f32, isOutput=False)
    bk_d = nc.declare_dram_parameter("bk", [CH], f32, isOutput=False)
    bv_d = nc.declare_dram_parameter("bv", [CH], f32, isOutput=False)
    bp_d = nc.declare_dram_parameter("bp", [OUTC], f32, isOutput=False)
    tri_d = nc.declare_dram_parameter("tri", [P, P], ddram, isOutput=False)
    out_d = nc.declare_dram_parameter("out", [T, OUTC], f32, isOutput=True)

    # softmax denominators: 32 chains of 512, bounced through DRAM so a
    # batched reciprocal can run across partitions. l in bf16 (one rounding,
    # <=2^-9 rel err on the denominator), reciprocals back in f32.
    l_d = nc.dram_tensor("l_i", [NCHAIN, GSZ], bf16)
    r_d = nc.dram_tensor("r_i", [NCHAIN, GSZ], f32)

    add = mybir.AluOpType.add
    mult = mybir.AluOpType.mult

    with tile.TileContext(nc) as tc:
        from contextlib import ExitStack
        with ExitStack() as ctx:
            persist = ctx.enter_context(tc.tile_pool(name="persist", bufs=1))
            qkin = ctx.enter_context(tc.tile_pool(name="qkin", bufs=2))
            ptile = ctx.enter_context(tc.tile_pool(name="ptile", bufs=6))
            rtile = ctx.enter_context(tc.tile_pool(name="rtile", bufs=4))
            outs = ctx.enter_context(tc.tile_pool(name="outs", bufs=3))
            pps = ctx.enter_context(tc.tile_pool(name="pps", bufs=1, space="PSUM"))
            sps = ctx.enter_context(tc.tile_pool(name="sps", bufs=2, space="PSUM"))
            yps = ctx.enter_context(tc.tile_pool(name="yps", bufs=1, space="PSUM"))

            # persistent SBUF tensors (x in 4 independent t-chunks so early
            # compute only waits on the first chunk's DMA)
            xts = [persist.tile([P, NKO, GSZ], din, name=f"xt{c}")
                   for c in range(4)]
            v_sb = persist.tile([P, NTT, NH, D + 1], din)    # V + ones col
            yT_sb = persist.tile([P, NM, T], din)            # attn out, [ch, t]
            wq_sb = persist.tile([P, NKO, CH], din)
            wk_sb = persist.tile([P, NKO, CH], din)
            wv_sb = persist.tile([P, NKO, CH], din)
            wpT_sb = persist.tile([P, NM, OUTC], din)
            tri_sb = persist.tile([P, P], ddram)
            bq_sb = persist.tile([P, NM], f32)
            bk_sb = persist.tile([P, NM], f32)
            bv_bc = persist.tile([P, CH], f32)
            bp_bc = persist.tile([P, OUTC], f32)

            # critical-path DMAs first: V needs x chunk 0 + Wv immediately;
            # both split by ko-half across queues so the first accumulation
            # can begin after half the data lands
            xT_r = xT_d[:].rearrange("(ko p) t -> p ko t", p=P)
            wv_r = wvT_d[:].rearrange("(ko p) c -> p ko c", p=P)
            nc.sync.dma_start(out=xts[0][:, 0:4, :], in_=xT_r[:, 0:4, 0:GSZ])
            nc.gpsimd.dma_start(out=wv_sb[:, 0:4, :], in_=wv_r[:, 0:4, :])
            nc.scalar.dma_start(out=xts[0][:, 4:8, :], in_=xT_r[:, 4:8, 0:GSZ])
            nc.sync.dma_start(out=wv_sb[:, 4:8, :], in_=wv_r[:, 4:8, :])
            nc.scalar.dma_start(out=xts[1][:], in_=xT_r[:, :, GSZ:2 * GSZ])
            nc.gpsimd.dma_start(out=xts[2][:], in_=xT_r[:, :, 2 * GSZ:3 * GSZ])
            nc.scalar.dma_start(out=xts[3][:], in_=xT_r[:, :, 3 * GSZ:4 * GSZ])
            nc.sync.dma_start(out=tri_sb[:], in_=tri_d[:])
            nc.sync.dma_start(out=bq_sb[:], in_=bq_d[:].rearrange("(m p) -> p m", p=P))
            nc.sync.dma_start(out=bk_sb[:], in_=bk_d[:].rearrange("(m p) -> p m", p=P))
            nc.sync.dma_start(out=wq_sb[:], in_=wqT_d[:].rearrange("(ko p) c -> p ko c", p=P))
            nc.gpsimd.dma_start(out=wk_sb[:], in_=wkT_d[:].rearrange("(ko p) c -> p ko c", p=P))
            nc.gpsimd.dma_start(out=bv_bc[:], in_=bv_d[None, :].to_broadcast((P, CH)))
            nc.gpsimd.dma_start(out=bp_bc[:], in_=bp_d[None, :].to_broadcast((P, OUTC)))
            nc.gpsimd.dma_start(out=wpT_sb[:], in_=wpT_d[:].rearrange("(m p) o -> p m o", p=P))
            ones_sb = persist.tile([P, NTT * NH], f32)
            nc.vector.memset(ones_sb[:], 1.0)
            nc.vector.tensor_copy(                           # ones columns in V
                out=v_sb[:, :, :, D],
                in_=ones_sb[:].rearrange("p (a b) -> p a b", b=NH))

            mask_eng = nc.vector

            # ---------------- V (one t-block = one unit) ----------
            # accumulations alternate between two PSUM banks: back-to-back
            # matmuls into the same bank serialize (~426ns vs 215ns spacing),
            # alternating banks pipelines fill/drain fully; halves are summed
            # at drain time on the DVE.
            def v_unit(tb):
                def unit():
                    xt = xts[tb // 4]
                    to = (tb % 4) * P
                    psa = pps.tile([P, CH], f32, tag="accA")
                    psb = pps.tile([P, CH], f32, tag="accB")
                    for ko in range(NKO):
                        nc.tensor.matmul(
                            (psa if ko % 2 == 0 else psb)[:],
                            xt[:, ko, to:to + P],
                            wv_sb[:, ko, :],
                            start=(ko < 2), stop=(ko >= NKO - 2),
                        )
                    nc.vector.tensor_tensor(
                        out=v_sb[:, tb, :, 0:D],
                        in0=psa[:].rearrange("p (h d) -> p h d", d=D),
                        in1=bv_bc[:].rearrange("p (h d) -> p h d", d=D),
                        op=add,
                    )
                    nc.vector.tensor_tensor(
                        out=v_sb[:, tb, :, 0:D],
                        in0=psb[:].rearrange("p (h d) -> p h d", d=D),
                        in1=v_sb[:, tb, :, 0:D],
                        op=add,
                    )
                return unit

            # first 4 t-blocks now (only need x chunk 0); rest are filler
            # units interleaved into group 0's attention
            for tb in range(4):
                v_unit(tb)()

            # ------------- q/k projection units (emitted lazily) -------------
            def make_proj(gn):
                """Allocate qg/kg tiles for group gn; return (tiles, unit fns)."""
                qg_n = qkin.tile([P, T], din, tag="qg", name=f"qg{gn}")
                kg_n = qkin.tile([P, T], din, tag="kg", name=f"kg{gn}")
                units = []
                for (w_sb, b_sb, dst) in ((wq_sb, bq_sb, qg_n), (wk_sb, bk_sb, kg_n)):
                    for ck in range(4):
                        def unit(w_sb=w_sb, b_sb=b_sb, dst=dst, ck=ck):
                            psa = pps.tile([P, GSZ], f32, tag="accA")
                            psb = pps.tile([P, GSZ], f32, tag="accB")
                            for ko in range(NKO):
                                nc.tensor.matmul(
                                    (psa if ko % 2 == 0 else psb)[:],
                                    w_sb[:, ko, gn * P:(gn + 1) * P],
                                    xts[ck][:, ko, :],
                                    start=(ko < 2), stop=(ko >= NKO - 2),
                                )
                            dsl = dst[:, ck * GSZ:(ck + 1) * GSZ]
                            nc.vector.tensor_scalar_add(
                                dsl, psa[:], b_sb[:, gn:gn + 1])
                            nc.vector.tensor_tensor(
                                out=dsl, in0=psb[:], in1=dsl, op=add)
                        units.append(unit)
            # interleaved: one unit every `step` attention j-iterations
                return (qg_n, kg_n), units

            (qg, kg), units0 = make_proj(0)
            for u in units0:
                u()

            # ------------- output-projection units (emitted lazily) -----------
            _oq = [nc.sync, nc.scalar, nc.gpsimd]

            def ph3_unit(ts, ih):
                def unit():
                    psa = pps.tile([P, 512], f32, tag="accA")
                    psb = pps.tile([P, 512], f32, tag="accB")
                    for co in range(NM):
                        nc.tensor.matmul(
                            (psa if co % 2 == 0 else psb)[:],
                            yT_sb[:, co, ts * P:(ts + 1) * P],
                            wpT_sb[:, co, ih * 512:(ih + 1) * 512],
                            start=(co < 2), stop=(co >= NM - 2),
                        )
                    ob = outs.tile([P, 512], f32, tag="ob")
                    nc.vector.tensor_tensor(
                        out=ob[:], in0=psa[:],
                        in1=bp_bc[:, ih * 512:(ih + 1) * 512], op=add)
                    nc.vector.tensor_tensor(
                        out=ob[:], in0=psb[:], in1=ob[:], op=add)
                    _oq[(ts * 2 + ih) % 3].dma_start(
                        out=out_d[ts * P:(ts + 1) * P, ih * 512:(ih + 1) * 512],
                        in_=ob[:])
                return unit

            ph3 = [ph3_unit(ts, ih) for ts in range(NTT) for ih in range(2)]

            # ---------------- per-group attention pipeline ----------------
            # gi-serial, hh row-packed: the two heads' S matmuls run
            # concurrently in PE row-groups 0-1 / 2-3 via tile_position, one
            # exp instruction covers both heads, and PV accumulates the two
            # heads' y in two PSUM banks.
            for g in range(NM):
                hA, hB = 2 * g, 2 * g + 1
                if g + 1 < NM:
                    (qg_next, kg_next), pend = make_proj(g + 1)
                else:
                    qg_next = kg_next = None
                    pend = []
                if g == 0:
                    # remaining V t-blocks lead the filler queue: attention
                    # consumes v[tb=j] so they must stay ahead of the j-loop
                    pend = [v_unit(tb) for tb in range(4, NTT)] + pend
                niter = sum((gi + 1) * 4 for gi in range(NG))   # 40
                step = max(1, niter // (len(pend) + 1)) if pend else niter + 1
                it = 0

                for gi in range(NG):
                    if g == NM - 1:
                        # cascade output-projection units for token groups
                        # whose normalization completed in earlier gi passes
                        if gi:
                            pend = pend + ph3[8 * (gi - 1):8 * gi]
                        step = 2
                    gst = gi * GSZ
                    nblk = (gst + GSZ) // P
                    y0 = yps.tile([D + 1, GSZ], f32, tag="y0",
                                  name=f"y0_{g}_{gi}")
                    y1 = yps.tile([D + 1, GSZ], f32, tag="y1",
                                  name=f"y1_{g}_{gi}")
                    # PV is software-pipelined one iteration behind S/exp so
                    # the PE never head-of-line-blocks waiting on exp.
                    pv_pending = None

                    def emit_pv(j, p2, dlt, nblk=nblk, y0=y0, y1=y1):
                        for hh, y, h in ((0, y0, hA), (1, y1, hB)):
                            nc.tensor.matmul(
                                y[:, dlt:],
                                v_sb[:, j, h, :],
                                p2[:, hh * GSZ + dlt:(hh + 1) * GSZ],
                                start=(j == 0), stop=(j == nblk - 1),
                            )

                    for j in range(nblk):
                        dlt = max(0, j * P - gst)
                        s2 = sps.tile([P, 2 * GSZ], f32, tag="s")
                        for lo in (0, D):
                            nc.tensor.matmul(
                                s2[:, (lo // D) * GSZ + dlt:
                                   (lo // D + 1) * GSZ],
                                kg[lo:lo + D, j * P:(j + 1) * P],
                                qg[lo:lo + D, gst + dlt:gst + GSZ],
                                start=True, stop=True, tile_position=(lo, 0),
                            )
                        p2 = ptile.tile([P, 2 * GSZ], din, tag="p")
                        if dlt:
                            nc.scalar.activation(
                                out=p2[:].rearrange(
                                    "p (h q) -> p h q", h=2)[:, :, dlt:],
                                in_=s2[:].rearrange(
                                    "p (h q) -> p h q", h=2)[:, :, dlt:],
                                func=mybir.ActivationFunctionType.Exp)
                        else:
                            nc.scalar.activation(
                                out=p2[:], in_=s2[:],
                                func=mybir.ActivationFunctionType.Exp)
                        if j * P >= gst:   # diagonal block: causal 0/1 mask
                            for hh in (0, 1):
                                sl = slice(hh * GSZ + dlt, hh * GSZ + dlt + P)
                                mask_eng.tensor_tensor(
                                    out=p2[:, sl], in0=p2[:, sl],
                                    in1=tri_sb[:], op=mult)
                        if pv_pending is not None:
                            emit_pv(*pv_pending)
                        pv_pending = (j, p2, dlt)
                        it += 1
                        if pend and it % step == 0:
                            pend.pop(0)()
                    emit_pv(*pv_pending)
                    # drain: one CAST releases each y accumulator; split
                    # y / denominator row from the staging copy
                    c0 = (g * NG + gi) * 2
                    for hh, yt in ((0, y0), (1, y1)):
                        lo = hh * D
                        ystg = rtile.tile([D + 1, GSZ], din, tag="ystg")
                        nc.vector.tensor_copy(out=ystg[:], in_=yt[:])
                        nc.vector.tensor_copy(
                            out=yT_sb[lo:lo + D, g, gst:gst + GSZ],
                            in_=ystg[0:D, :])
                        nc.sync.dma_start(out=l_d[c0 + hh:c0 + hh + 1, :],
                                          in_=ystg[D:D + 1, :])
                    # per-gi reciprocal + normalize (overlaps later work)
                    lp = rtile.tile([8, P], bf16, tag="lp")
                    rp = rtile.tile([8, P], f32, tag="rp")
                    nc.sync.dma_start(
                        out=lp[:], in_=l_d[c0:c0 + 2, :].rearrange(
                            "c (a b) -> (c a) b", b=P))
                    nc.vector.reciprocal(rp[:], lp[:])
                    nc.sync.dma_start(
                        out=r_d[c0:c0 + 2, :].rearrange("c (a b) -> (c a) b",
                                                        b=P),
                        in_=rp[:])
                    for hh in (0, 1):
                        lo = hh * D
                        rb = rtile.tile([P, GSZ], f32, tag="rb")
                        nc.sync.dma_start(
                            out=rb[lo:lo + D, :],
                            in_=r_d[c0 + hh:c0 + hh + 1, :]
                            .to_broadcast((D, GSZ)))
                        ysl = yT_sb[lo:lo + D, g, gst:gst + GSZ]
                        nc.vector.tensor_tensor(
                            out=ysl, in0=ysl, in1=rb[lo:lo + D, :], op=mult)
                while pend:
                    pend.pop(0)()
                if g == NM - 1:
                    for u in ph3[24:]:
                        u()

                qg, kg = qg_next, kg_next

    nc.compile()
    return nc


def make_in_maps(x, Wq, bq, Wk, bk, Wv, bv, Wp, bp, dt_mode="bf16"):
    """Shard full inputs into 8 per-core input maps."""
    import ml_dtypes
    npdt = ml_dtypes.bfloat16 if dt_mode == "bf16" else np.float32
    x = np.asarray(x, np.float32)
    scale = 1.0 / np.sqrt(D)
    tri = np.where(np.arange(P)[:, None] > np.arange(P)[None, :], 0.0, 1.0).astype(npdt)
    zeros_bp = np.zeros(OUTC, np.float32)
    in_maps = []
    for c in range(8):
        b, half = divmod(c, 2)
        sl = slice(half * CH, (half + 1) * CH)
        in_maps.append({
            "xT": np.ascontiguousarray(x[b].T).astype(npdt),
            "wqT": np.ascontiguousarray((np.asarray(Wq, np.float32)[sl] * scale).T).astype(npdt),
            "wkT": np.ascontiguousarray(np.asarray(Wk, np.float32)[sl].T).astype(npdt),
            "wvT": np.ascontiguousarray(np.asarray(Wv, np.float32)[sl].T).astype(npdt),
            "wpT": np.ascontiguousarray(np.asarray(Wp, np.float32)[:, sl].T).astype(npdt),
            "bq": (np.asarray(bq, np.float32)[sl] * scale).copy(),
            "bk": np.asarray(bk, np.float32)[sl].copy(),
            "bv": np.asarray(bv, np.float32)[sl].copy(),
            "bp": np.asarray(bp, np.float32).copy() if half == 0 else zeros_bp,
            "tri": tri,
        })
    return in_maps


def combine(results):
    """results: list of 8 dicts with 'out' [T, OUTC] partials -> [4, T, OUTC]."""
    return np.stack([results[2 * b]["out"] + results[2 * b + 1]["out"]
                     for b in range(4)]).astype(np.float32)


# ----------------------------------------------------------------------------
# Harness entry point: full inputs in, full output out.
# ----------------------------------------------------------------------------
_NC_CACHE = {}


def _get_nc(dt_mode):
    if dt_mode not in _NC_CACHE:
        _NC_CACHE[dt_mode] = build_nc(dt_mode)
    return _NC_CACHE[dt_mode]


def kernel(x, Wq, bq, Wk, bk, Wv, bv, Wp, bp):
    from concourse.bass_utils import run_bass_kernel_spmd
    dt_mode = "bf16"
    nc = _get_nc(dt_mode)
    in_maps = make_in_maps(x, Wq, bq, Wk, bk, Wv, bv, Wp, bp, dt_mode)
    res = run_bass_kernel_spmd(nc, in_maps, list(range(8)))
    return combine(res.results)
